# revision 12
# baseline (speedup 1.0000x reference)
"""Trainium2 Bass kernel: SNN Leaky-Integrate-and-Fire layer.

Computes, for x [T=1024, N_IN=4096] f32 and W [N_OUT=4096, N_IN=4096] f32:
    cur = x @ W.T                                   # [T, N_OUT]
    mem_t = 0.9*mem_{t-1} + cur_t - (mem_{t-1} > 1)  # scan over T
    spk_t = (mem_t > 1)
returning (spk_rec, mem_rec), both [T, N_OUT] f32.

Sharding: N_OUT split across 8 NeuronCores (512 neurons each). x is
replicated (each core reads the T-half slices it needs exactly once).

Device algorithm per core (neurons on partitions, time along free dim):
  cur[o, t] accumulated in PSUM via 256 matmuls (K=4096 in 32 tiles,
  O=512 in 4 tiles of 128 partitions, T=1024 in 2 half-banks of 512).
  The scan is decomposed as mem = A + B with
     A_t = 0.9*A_{t-1} + cur_t        (pure linear -> one HW
                                       tensor_tensor_scan per 512 steps)
     B_t = 0.9*B_{t-1} - s_{t-1}      (spike-coupled residual)
     s_t = (B_t > theta_t),  theta_t = 1 - A_t   (precomputed in bulk)
  so the serial part is only 2 small DVE instructions per timestep.
"""

import numpy as np

T = 1024
N_IN = 4096
N_OUT = 4096
N_CORES = 8
O_SHARD = N_OUT // N_CORES  # 512
KT = N_IN // 128  # 32 k-tiles
OT = O_SHARD // 128  # 4 o-tiles
BETA = 0.9
THRESHOLD = 1.0

_CACHE = {}

LIF_OP_NAME = "LIF_SCAN_ANT"


def _build_lif_uops():
    """DVE uop program for the LIF recurrence:

        m_i = beta*m_{i-1} + c_i - (m_{i-1} > 1)
        (beta = CONST_0/s0, init m = CONST_1/s1); out[i] = m_i, len = N.

    3 cycles/element, structured like the silicon-validated stock
    TENSOR_TENSOR_SCAN (seed -> bubbles -> consuming steady uop, state in
    a-flops read via NEXT_ALU_OUT_A one cycle after the write):

      E  (consume, cycle 3i):   blk0: w = SUB(c_i, blk1.a = r_{i-1})
                  (cycle 3i+1): blk1: m_i = ADD(w, blk2.a = u_i); out flop
                                blk2..7: BYPASS chain -> WR0_LO
      Bx (bubble, cycle 3i+2):  blk1: r_i = IS_GT(CURR_ALU_OUT = m_i, 1)
                                      -> out+a flop
                  (cycle 3i+3): blk2: u_{i+1} = MULT(CURR_ALU_OUT = m_i, beta)
                                      -> out+a flop
      By (bubble):              inert; gives blk2.a write one cycle to land
                                before E's blk1 read.

    Seed S0 (stock-seed clone) parks init m in blk1's out flop so the first
    Bx computes r/u from it. All cross-element state rides a-flops and
    CURR_ALU_OUT temporal reads; SRC_0 is only touched by the consuming uop."""
    from concourse.dve_uop import (
        ENABLE,
        AluInp,
        AluOp,
        InpSel,
        OutPath,
        OutSel,
        Trigger,
        UopConfig,
    )

    CNT, NONE, SRC_DONE = Trigger.COUNT, Trigger.NONE, Trigger.SRC_TENSOR_DONE

    # 0: S0 seed — blk1.out <- 0 (hard-wired ZERO; CONST_1 is not delivered
    # to the datapath on non-consuming uops on this firmware, so chunk
    # carry-in is handled by prepending m_prev as an input column instead)
    s0 = UopConfig()
    s0.enable_input(InpSel.CONST_1, 1)
    s0.datapath_config[0].enable_alu(AluOp.BYPASS, AluInp.PREV_DELAY_0)
    s0.datapath_config[1].pass_through_alu()
    s0.datapath_config[1].alu_out_a_enable = ENABLE
    # blk2's out flop seeds Bx's u = beta*m read (CURR_ALU_OUT at blk2)
    s0.datapath_config[2].pass_through_alu()
    s0.trigger = (CNT, NONE, NONE)
    s0.next_uop = (1, 0, 0)
    s0.repeat_count = 1

    # 1: Bx working bubble — r and u from the m in blk1's out flop
    bx = UopConfig()
    bx.enable_input(InpSel.ONE_F32, 2)  # -> delay_1 (threshold)
    bx.enable_input(InpSel.CONST_0, 3)  # -> delay_2 (beta)
    bx.datapath_config[0].pass_through_delay(1, 2)
    bx.datapath_config[1].enable_alu(
        AluOp.IS_GT, AluInp.CURR_ALU_OUT, AluInp.PREV_DELAY_1
    )
    bx.datapath_config[1].alu_out_a_enable = ENABLE
    bx.datapath_config[1].pass_through_delay(2)
    bx.datapath_config[2].enable_alu(
        AluOp.MULTIPLY, AluInp.CURR_ALU_OUT, AluInp.PREV_DELAY_2
    )
    bx.datapath_config[2].alu_out_a_enable = ENABLE
    bx.trigger = (CNT, NONE, NONE)
    bx.next_uop = (2, 0, 0)
    bx.repeat_count = 1

    # 2: By inert bubble
    by = UopConfig()
    by.trigger = (CNT, NONE, NONE)
    by.next_uop = (3, 0, 0)
    by.repeat_count = 1

    # 3: E steady — consumes c, emits m
    e = UopConfig()
    e.enable_input(InpSel.SRC_0, 0)
    e.require_inp0 = ENABLE
    e.datapath_config[0].enable_alu(
        AluOp.SUBTRACT, AluInp.PREV_ALU_OUT, AluInp.NEXT_ALU_OUT_A
    )
    e.datapath_config[1].enable_alu(
        AluOp.ADD, AluInp.PREV_ALU_OUT, AluInp.NEXT_ALU_OUT_A
    )
    for b in range(2, 8):
        e.datapath_config[b].pass_through_alu()
    e.enable_output(OutSel.ALU_OUT, OutPath.WR0_LO)
    e.trigger = (SRC_DONE, CNT, NONE)
    e.next_uop = (0, 1, 0)
    e.repeat_count = 1

    uops = [s0, bx, by, e]
    for u in uops:
        u.validate("v3")
    return uops


def _register_lif_op():
    import numpy as np_

    import concourse.dve_ops as dve_ops
    from concourse.dve_ops import DveOp
    from concourse.dve_spec import Spec, Src0
    from concourse.dve_uop import DveOpSpec

    if LIF_OP_NAME in dve_ops._SUB_OPCODE_FOR_NAME:
        return _CACHE["lif_op"]

    def _ref(in0, in1, c0, c1, c2):
        out = np_.empty((in0.shape[0], in0.shape[1]), np_.float32)
        m = np_.full((in0.shape[0],), c1, np_.float32)
        for t in range(in0.shape[1]):
            r = (m > 1.0).astype(np_.float32)
            m = np_.float32(c0) * m + (in0[:, t] - r)
            out[:, t] = m
        return out

    class _RawDveOp(DveOp):
        def compile(self, ver):
            assert ver == "v3", "LIF_SCAN_ANT is v3/TRN2-only"
            return DveOpSpec(
                name=self.name,
                opcode=dve_ops.get_dve_sub_opcode(self.name),
                uops=_build_lif_uops(),
                rd1_en=False,
            )

    op = _RawDveOp(
        name=LIF_OP_NAME,
        spec=Spec(body=Src0, reference=_ref),
        subdim=False,
        uops_sha={},
    )
    dve_ops.OPS.append(op)
    dve_ops._SUB_OPCODE_FOR_NAME[op.name] = (
        dve_ops._CUSTOM_DVE_ROW_BASE + len(dve_ops.OPS) - 1
    )
    dve_ops.CUSTOM_DVE_SPECS[op.name] = op.spec
    _CACHE["lif_op"] = op
    return op


def _build_nc(mm_dtype_name: str):
    import concourse.bacc as bacc
    import concourse.mybir as mybir
    from concourse.tile import TileContext

    F32 = mybir.dt.float32
    custom = mm_dtype_name in ("custom", "cf32r")
    split3 = mm_dtype_name in ("split3", "custom")
    MMDT = (
        mybir.dt.bfloat16
        if split3
        else mybir.dt.float32r
        if mm_dtype_name == "cf32r"
        else getattr(mybir.dt, mm_dtype_name)
    )
    Op = mybir.AluOpType
    lif_op = _register_lif_op() if custom else None

    nc = bacc.Bacc(target_bir_lowering=False)
    # split3: xT/WT carry [2, ...] leading dim = (hi, lo) bf16 parts.
    xshape = [2, N_IN, T] if split3 else [N_IN, T]
    wshape = [2, N_IN, O_SHARD] if split3 else [N_IN, O_SHARD]
    xT_d = nc.dram_tensor("xT", xshape, MMDT, kind="ExternalInput")
    WT_d = nc.dram_tensor("WT", wshape, MMDT, kind="ExternalInput")
    spk_d = nc.dram_tensor("spk", [O_SHARD, T], F32, kind="ExternalOutput")
    mem_d = nc.dram_tensor("mem", [O_SHARD, T], F32, kind="ExternalOutput")

    with TileContext(nc) as tc:
        with (
            tc.tile_pool(name="sb", bufs=1) as sb,
            tc.tile_pool(name="xs", bufs=4) as xs,
            tc.tile_pool(name="psp", bufs=1, space="PSUM") as psp,
        ):
            # All weights resident: [128, 2|1, KT, O_SHARD]; k-tile k holds
            # WT rows k*128..k*128+127 (i.e. W.T), so wt[:, h, k, o*128:...]
            # is directly the matmul stationary operand [K=128, M=128].
            # W streams on the Activation HWDGE ring so x DMAs (SP ring)
            # are not queued behind it.
            NH = 2 if split3 else 1
            wt = sb.tile([128, NH, KT, O_SHARD], MMDT, name="wt")
            if split3:
                wt_view = WT_d.rearrange("h (k p) o -> p h k o", p=128)
            else:
                wt_view = WT_d.rearrange("(k p) o -> p () k o", p=128)
            w_eng = nc.scalar if custom else nc.sync
            for h in range(NH):
                for kc in range(0, KT, 4):
                    w_eng.dma_start(
                        wt[:, h, kc : kc + 4, :], wt_view[:, h, kc : kc + 4, :]
                    )

            if custom:
                # one PSUM bank per (o, th) chunk so th1 matmuls never wait
                # on th0 scans through a shared-tile dependency
                psc = [
                    [
                        psp.tile([128, 512], F32, name=f"ps{o}_{th}", tag=f"ps{o}_{th}")
                        for th in range(2)
                    ]
                    for o in range(OT)
                ]
                ps = None
            else:
                ps = [
                    psp.tile([128, T], F32, name=f"ps{o}", tag=f"ps{o}")
                    for o in range(OT)
                ]

            if custom:
                M = sb.tile([128, OT, T], F32, name="M")
                Sp = sb.tile([128, OT, T], F32, name="Sp")
            else:
                A = sb.tile([128, OT, T], F32, name="A")  # linear-part scan
                TH = sb.tile([128, OT, T], F32, name="TH")  # theta = 1 - A
                M = sb.tile([128, OT, T], F32, name="M")  # mem = A + B
                Bb = sb.tile([128, OT, T + 1], F32, name="Bb")  # residual state
                Sb = sb.tile([128, OT, T + 1], F32, name="Sb")  # spikes (0/1)
                beta_t = sb.tile([128, 512], F32, name="beta_t")

                nc.vector.memset(beta_t, BETA)
                nc.vector.memset(Bb[:, :, 0], 0.0)
                nc.vector.memset(Sb[:, :, 0], 0.0)

            for th in range(2):
                tl, tr = th * 512, (th + 1) * 512
                # ---- matmul: accumulate cur[:, tl:tr] over all K ----
                for k in range(KT):
                    if split3:
                        xh = xs.tile([128, 2, 512], MMDT, name="xh")
                        nc.sync.dma_start(
                            xh,
                            xT_d[:, k * 128 : (k + 1) * 128, tl:tr].rearrange(
                                "h p t -> p h t"
                            ),
                        )
                        terms = [(0, 0), (1, 0), (0, 1)]  # (h_w, h_x)
                    else:
                        xh = xs.tile([128, 1, 512], MMDT, name="xh")
                        nc.sync.dma_start(
                            xh[:, 0, :], xT_d[k * 128 : (k + 1) * 128, tl:tr]
                        )
                        terms = [(0, 0)]
                    for o in range(OT):
                        tgt = psc[o][th] if custom else ps[o][:, tl:tr]
                        for ti, (hw, hx) in enumerate(terms):
                            nc.tensor.matmul(
                                tgt,
                                lhsT=wt[:, hw, k, o * 128 : (o + 1) * 128],
                                rhs=xh[:, hx, :],
                                start=(k == 0 and ti == 0),
                                stop=(k == KT - 1 and ti == len(terms) - 1),
                            )
                if custom:
                    for o in range(OT):
                        init = 0.0 if th == 0 else M[:, o, tl - 1 : tl]
                        nc.vector._custom_dve(
                            lif_op,
                            out=M[:, o, tl:tr],
                            in0=psc[o][th],
                            s0=BETA,
                            s1=init,
                        )
                        nc.gpsimd.tensor_scalar(
                            Sp[:, o, tl:tr],
                            M[:, o, tl:tr],
                            1.0,
                            None,
                            Op.is_gt,
                        )
                        nc.scalar.dma_start(
                            spk_d[o * 128 : (o + 1) * 128, tl:tr], Sp[:, o, tl:tr]
                        )
                        nc.scalar.dma_start(
                            mem_d[o * 128 : (o + 1) * 128, tl:tr],
                            M[:, o, tl:tr],
                        )
                    continue
                # ---- bulk prep for this half: A scan + theta ----
                for o in range(OT):
                    init = 0.0 if th == 0 else A[:, o, tl - 1 : tl]
                    nc.vector.tensor_tensor_scan(
                        out=A[:, o, tl:tr],
                        data0=beta_t,
                        data1=ps[o][:, tl:tr],
                        initial=init,
                        op0=Op.mult,
                        op1=Op.add,
                    )
                    nc.gpsimd.tensor_scalar(
                        TH[:, o, tl:tr], A[:, o, tl:tr], -1.0, THRESHOLD, Op.mult, Op.add
                    )
                # ---- serial scan for this half: 2 DVE instrs per step ----
                for t in range(tl + 1, tr + 1):
                    nc.vector.scalar_tensor_tensor(
                        out=Bb[:, :, t],
                        in0=Bb[:, :, t - 1],
                        scalar=BETA,
                        in1=Sb[:, :, t - 1],
                        op0=Op.mult,
                        op1=Op.subtract,
                    )
                    nc.vector.tensor_tensor(
                        Sb[:, :, t], Bb[:, :, t], TH[:, :, t - 1], Op.is_gt
                    )
                # ---- epilogue for this half: mem = A + B, DMA out ----
                for o in range(OT):
                    nc.gpsimd.tensor_tensor(
                        M[:, o, tl:tr], A[:, o, tl:tr], Bb[:, o, tl + 1 : tr + 1], Op.add
                    )
                    nc.sync.dma_start(
                        spk_d[o * 128 : (o + 1) * 128, tl:tr], Sb[:, o, tl + 1 : tr + 1]
                    )
                    nc.sync.dma_start(
                        mem_d[o * 128 : (o + 1) * 128, tl:tr], M[:, o, tl:tr]
                    )
    nc.finalize()
    return nc


def _get_nc(mm_dtype_name: str):
    if mm_dtype_name not in _CACHE:
        _CACHE[mm_dtype_name] = _build_nc(mm_dtype_name)
    return _CACHE[mm_dtype_name]


def run(x, W, mm_dtype_name="split3", trace=False):
    import ml_dtypes

    from concourse.bass_utils import run_bass_kernel_spmd

    bf16 = ml_dtypes.bfloat16
    nc = _get_nc(mm_dtype_name)
    x = np.asarray(x, dtype=np.float32)
    W = np.asarray(W, dtype=np.float32)
    in_maps = []
    if mm_dtype_name in ("split3", "custom"):
        x_hi = x.astype(bf16)
        x_lo = (x - x_hi.astype(np.float32)).astype(bf16)
        xT = np.ascontiguousarray(
            np.stack([x_hi.T, x_lo.T], axis=0)
        )  # [2, N_IN, T] bf16
        W_hi = W.astype(bf16)
        W_lo = (W - W_hi.astype(np.float32)).astype(bf16)
        for c in range(N_CORES):
            sl = slice(c * O_SHARD, (c + 1) * O_SHARD)
            WTc = np.ascontiguousarray(np.stack([W_hi[sl].T, W_lo[sl].T], axis=0))
            in_maps.append({"xT": xT, "WT": WTc})
    else:
        xT = np.ascontiguousarray(x.T)  # [N_IN, T]
        for c in range(N_CORES):
            WTc = np.ascontiguousarray(W[c * O_SHARD : (c + 1) * O_SHARD, :].T)
            in_maps.append({"xT": xT, "WT": WTc})
    res = run_bass_kernel_spmd(nc, in_maps, core_ids=list(range(N_CORES)), trace=trace)
    spk = np.concatenate([r["spk"] for r in res.results], axis=0).T
    mem = np.concatenate([r["mem"] for r in res.results], axis=0).T
    return (
        np.ascontiguousarray(spk),
        np.ascontiguousarray(mem),
    ), res


def kernel(x, W):
    out, _ = run(x, W)
    return out



# revision 15
# speedup vs baseline: 1.3236x; 1.3236x over previous
"""Trainium2 Bass kernel: SNN Leaky-Integrate-and-Fire layer.

Computes, for x [T=1024, N_IN=4096] f32 and W [N_OUT=4096, N_IN=4096] f32:
    cur = x @ W.T                                   # [T, N_OUT]
    mem_t = 0.9*mem_{t-1} + cur_t - (mem_{t-1} > 1)  # scan over T
    spk_t = (mem_t > 1)
returning (spk_rec, mem_rec), both [T, N_OUT] f32.

Sharding: N_OUT split across 8 NeuronCores (512 neurons each). x is
replicated (each core reads the T-half slices it needs exactly once).

Device algorithm per core (neurons on partitions, time along free dim):
  cur[o, t] accumulated in PSUM via 256 matmuls (K=4096 in 32 tiles,
  O=512 in 4 tiles of 128 partitions, T=1024 in 2 half-banks of 512).
  The scan is decomposed as mem = A + B with
     A_t = 0.9*A_{t-1} + cur_t        (pure linear -> one HW
                                       tensor_tensor_scan per 512 steps)
     B_t = 0.9*B_{t-1} - s_{t-1}      (spike-coupled residual)
     s_t = (B_t > theta_t),  theta_t = 1 - A_t   (precomputed in bulk)
  so the serial part is only 2 small DVE instructions per timestep.
"""

import numpy as np

T = 1024
N_IN = 4096
N_OUT = 4096
N_CORES = 8
O_SHARD = N_OUT // N_CORES  # 512
KT = N_IN // 128  # 32 k-tiles
OT = O_SHARD // 128  # 4 o-tiles
BETA = 0.9
THRESHOLD = 1.0

_CACHE = {}

LIF_OP_NAME = "LIF_SCAN_ANT"


def _build_lif_uops():
    """DVE uop program for the LIF recurrence:

        m_i = beta*m_{i-1} + c_i - (m_{i-1} > 1)
        (beta = CONST_0/s0, init m = CONST_1/s1); out[i] = m_i, len = N.

    3 cycles/element, structured like the silicon-validated stock
    TENSOR_TENSOR_SCAN (seed -> bubbles -> consuming steady uop, state in
    a-flops read via NEXT_ALU_OUT_A one cycle after the write):

      E  (consume, cycle 3i):   blk0: w = SUB(c_i, blk1.a = r_{i-1})
                  (cycle 3i+1): blk1: m_i = ADD(w, blk2.a = u_i); out flop
                                blk2..7: BYPASS chain -> WR0_LO
      Bx (bubble, cycle 3i+2):  blk1: r_i = IS_GT(CURR_ALU_OUT = m_i, 1)
                                      -> out+a flop
                  (cycle 3i+3): blk2: u_{i+1} = MULT(CURR_ALU_OUT = m_i, beta)
                                      -> out+a flop
      By (bubble):              inert; gives blk2.a write one cycle to land
                                before E's blk1 read.

    Seed S0 (stock-seed clone) parks init m in blk1's out flop so the first
    Bx computes r/u from it. All cross-element state rides a-flops and
    CURR_ALU_OUT temporal reads; SRC_0 is only touched by the consuming uop."""
    from concourse.dve_uop import (
        ENABLE,
        AluInp,
        AluOp,
        InpSel,
        OutPath,
        OutSel,
        Trigger,
        UopConfig,
    )

    CNT, NONE, SRC_DONE = Trigger.COUNT, Trigger.NONE, Trigger.SRC_TENSOR_DONE

    # 0: S0 seed — blk1.out <- 0 (hard-wired ZERO; CONST_1 is not delivered
    # to the datapath on non-consuming uops on this firmware, so chunk
    # carry-in is handled by prepending m_prev as an input column instead)
    s0 = UopConfig()
    s0.enable_input(InpSel.CONST_1, 1)
    s0.datapath_config[0].enable_alu(AluOp.BYPASS, AluInp.PREV_DELAY_0)
    s0.datapath_config[1].pass_through_alu()
    s0.datapath_config[1].alu_out_a_enable = ENABLE
    # blk2's out flop seeds Bx's u = beta*m read (CURR_ALU_OUT at blk2)
    s0.datapath_config[2].pass_through_alu()
    s0.trigger = (CNT, NONE, NONE)
    s0.next_uop = (1, 0, 0)
    s0.repeat_count = 1

    # 1: Bx working bubble — r and u from the m in blk1's out flop
    bx = UopConfig()
    bx.enable_input(InpSel.ONE_F32, 2)  # -> delay_1 (threshold)
    bx.enable_input(InpSel.CONST_0, 3)  # -> delay_2 (beta)
    bx.datapath_config[0].pass_through_delay(1, 2)
    bx.datapath_config[1].enable_alu(
        AluOp.IS_GT, AluInp.CURR_ALU_OUT, AluInp.PREV_DELAY_1
    )
    bx.datapath_config[1].alu_out_a_enable = ENABLE
    bx.datapath_config[1].pass_through_delay(2)
    bx.datapath_config[2].enable_alu(
        AluOp.MULTIPLY, AluInp.CURR_ALU_OUT, AluInp.PREV_DELAY_2
    )
    bx.datapath_config[2].alu_out_a_enable = ENABLE
    bx.trigger = (CNT, NONE, NONE)
    bx.next_uop = (2, 0, 0)
    bx.repeat_count = 1

    # 2: By inert bubble
    by = UopConfig()
    by.trigger = (CNT, NONE, NONE)
    by.next_uop = (3, 0, 0)
    by.repeat_count = 1

    # 3: E steady — consumes c, emits m
    e = UopConfig()
    e.enable_input(InpSel.SRC_0, 0)
    e.require_inp0 = ENABLE
    e.datapath_config[0].enable_alu(
        AluOp.SUBTRACT, AluInp.PREV_ALU_OUT, AluInp.NEXT_ALU_OUT_A
    )
    e.datapath_config[1].enable_alu(
        AluOp.ADD, AluInp.PREV_ALU_OUT, AluInp.NEXT_ALU_OUT_A
    )
    for b in range(2, 8):
        e.datapath_config[b].pass_through_alu()
    e.enable_output(OutSel.ALU_OUT, OutPath.WR0_LO)
    e.trigger = (SRC_DONE, CNT, NONE)
    e.next_uop = (0, 1, 0)
    e.repeat_count = 1

    uops = [s0, bx, by, e]
    for u in uops:
        u.validate("v3")
    return uops


def _register_lif_op():
    import numpy as np_

    import concourse.dve_ops as dve_ops
    from concourse.dve_ops import DveOp
    from concourse.dve_spec import Spec, Src0
    from concourse.dve_uop import DveOpSpec

    if LIF_OP_NAME in dve_ops._SUB_OPCODE_FOR_NAME:
        return _CACHE["lif_op"]

    def _ref(in0, in1, c0, c1, c2):
        out = np_.empty((in0.shape[0], in0.shape[1]), np_.float32)
        m = np_.full((in0.shape[0],), c1, np_.float32)
        for t in range(in0.shape[1]):
            r = (m > 1.0).astype(np_.float32)
            m = np_.float32(c0) * m + (in0[:, t] - r)
            out[:, t] = m
        return out

    class _RawDveOp(DveOp):
        def compile(self, ver):
            assert ver == "v3", "LIF_SCAN_ANT is v3/TRN2-only"
            return DveOpSpec(
                name=self.name,
                opcode=dve_ops.get_dve_sub_opcode(self.name),
                uops=_build_lif_uops(),
                rd1_en=False,
            )

    op = _RawDveOp(
        name=LIF_OP_NAME,
        spec=Spec(body=Src0, reference=_ref),
        subdim=False,
        uops_sha={},
    )
    dve_ops.OPS.append(op)
    dve_ops._SUB_OPCODE_FOR_NAME[op.name] = (
        dve_ops._CUSTOM_DVE_ROW_BASE + len(dve_ops.OPS) - 1
    )
    dve_ops.CUSTOM_DVE_SPECS[op.name] = op.spec
    _CACHE["lif_op"] = op
    return op


def _build_nc(mm_dtype_name: str):
    import concourse.bacc as bacc
    import concourse.mybir as mybir
    from concourse.tile import TileContext

    F32 = mybir.dt.float32
    custom = mm_dtype_name in ("custom", "cf32r")
    split3 = mm_dtype_name in ("split3", "custom")
    MMDT = (
        mybir.dt.bfloat16
        if split3
        else mybir.dt.float32r
        if mm_dtype_name == "cf32r"
        else getattr(mybir.dt, mm_dtype_name)
    )
    Op = mybir.AluOpType
    lif_op = _register_lif_op() if custom else None

    nc = bacc.Bacc(target_bir_lowering=False)
    # split3: xT/WT carry [2, ...] leading dim = (hi, lo) bf16 parts.
    xshape = [2, N_IN, T] if split3 else [N_IN, T]
    wshape = [2, N_IN, O_SHARD] if split3 else [N_IN, O_SHARD]
    xT_d = nc.dram_tensor("xT", xshape, MMDT, kind="ExternalInput")
    WT_d = nc.dram_tensor("WT", wshape, MMDT, kind="ExternalInput")
    spk_d = nc.dram_tensor("spk", [O_SHARD, T], F32, kind="ExternalOutput")
    mem_d = nc.dram_tensor("mem", [O_SHARD, T], F32, kind="ExternalOutput")

    with TileContext(nc) as tc:
        with (
            tc.tile_pool(name="sb", bufs=1) as sb,
            tc.tile_pool(name="xs", bufs=4) as xs,
            tc.tile_pool(name="psp", bufs=1, space="PSUM") as psp,
        ):
            # All weights resident: [128, 2|1, KT, O_SHARD]; k-tile k holds
            # WT rows k*128..k*128+127 (i.e. W.T), so wt[:, h, k, o*128:...]
            # is directly the matmul stationary operand [K=128, M=128].
            # W streams on the Activation HWDGE ring so x DMAs (SP ring)
            # are not queued behind it.
            NH = 2 if split3 else 1
            if split3:
                wt_view = WT_d.rearrange("h (k p) o -> p h k o", p=128)
            else:
                wt_view = WT_d.rearrange("(k p) o -> p () k o", p=128)
            w_eng = nc.scalar if custom else nc.sync
            # one tile per 4-k-tile DMA chunk: dependency granularity matches
            # the transfer, so matmul k waits only on its own chunk
            wchunks = [
                sb.tile([128, NH, 4, O_SHARD], MMDT, name=f"wt{kc}")
                for kc in range(0, KT, 4)
            ]
            for h in range(NH):
                for ci, kc in enumerate(range(0, KT, 4)):
                    w_eng.dma_start(
                        wchunks[ci][:, h, :, :], wt_view[:, h, kc : kc + 4, :]
                    )

            def wt_sl(h, k, o):
                return wchunks[k // 4][:, h, k % 4, o * 128 : (o + 1) * 128]

            if custom:
                # one PSUM bank per (o, th) chunk so th1 matmuls never wait
                # on th0 scans through a shared-tile dependency
                psc = [
                    [
                        psp.tile([128, 512], F32, name=f"ps{o}_{th}", tag=f"ps{o}_{th}")
                        for th in range(2)
                    ]
                    for o in range(OT)
                ]
                ps = None
            else:
                ps = [
                    psp.tile([128, T], F32, name=f"ps{o}", tag=f"ps{o}")
                    for o in range(OT)
                ]

            if custom:
                M = sb.tile([128, OT, T], F32, name="M")
                Sp = sb.tile([128, OT, T], F32, name="Sp")
            else:
                A = sb.tile([128, OT, T], F32, name="A")  # linear-part scan
                TH = sb.tile([128, OT, T], F32, name="TH")  # theta = 1 - A
                M = sb.tile([128, OT, T], F32, name="M")  # mem = A + B
                Bb = sb.tile([128, OT, T + 1], F32, name="Bb")  # residual state
                Sb = sb.tile([128, OT, T + 1], F32, name="Sb")  # spikes (0/1)
                beta_t = sb.tile([128, 512], F32, name="beta_t")

                nc.vector.memset(beta_t, BETA)
                nc.vector.memset(Bb[:, :, 0], 0.0)
                nc.vector.memset(Sb[:, :, 0], 0.0)

            for th in range(2):
                tl, tr = th * 512, (th + 1) * 512
                # ---- matmul: accumulate cur[:, tl:tr] over all K ----
                for k in range(KT):
                    if split3:
                        xh = xs.tile([128, 2, 512], MMDT, name="xh")
                        nc.sync.dma_start(
                            xh,
                            xT_d[:, k * 128 : (k + 1) * 128, tl:tr].rearrange(
                                "h p t -> p h t"
                            ),
                        )
                        terms = [(0, 0), (1, 0), (0, 1)]  # (h_w, h_x)
                    else:
                        xh = xs.tile([128, 1, 512], MMDT, name="xh")
                        nc.sync.dma_start(
                            xh[:, 0, :], xT_d[k * 128 : (k + 1) * 128, tl:tr]
                        )
                        terms = [(0, 0)]
                    for o in range(OT):
                        tgt = psc[o][th] if custom else ps[o][:, tl:tr]
                        for ti, (hw, hx) in enumerate(terms):
                            nc.tensor.matmul(
                                tgt,
                                lhsT=wt_sl(hw, k, o),
                                rhs=xh[:, hx, :],
                                start=(k == 0 and ti == 0),
                                stop=(k == KT - 1 and ti == len(terms) - 1),
                            )
                if custom:
                    for o in range(OT):
                        init = 0.0 if th == 0 else M[:, o, tl - 1 : tl]
                        nc.vector._custom_dve(
                            lif_op,
                            out=M[:, o, tl:tr],
                            in0=psc[o][th],
                            s0=BETA,
                            s1=init,
                        )
                        nc.vector.tensor_scalar(
                            Sp[:, o, tl:tr],
                            M[:, o, tl:tr],
                            1.0,
                            None,
                            Op.is_gt,
                        )
                        nc.scalar.dma_start(
                            spk_d[o * 128 : (o + 1) * 128, tl:tr], Sp[:, o, tl:tr]
                        )
                        nc.scalar.dma_start(
                            mem_d[o * 128 : (o + 1) * 128, tl:tr],
                            M[:, o, tl:tr],
                        )
                    continue
                # ---- bulk prep for this half: A scan + theta ----
                for o in range(OT):
                    init = 0.0 if th == 0 else A[:, o, tl - 1 : tl]
                    nc.vector.tensor_tensor_scan(
                        out=A[:, o, tl:tr],
                        data0=beta_t,
                        data1=ps[o][:, tl:tr],
                        initial=init,
                        op0=Op.mult,
                        op1=Op.add,
                    )
                    nc.gpsimd.tensor_scalar(
                        TH[:, o, tl:tr], A[:, o, tl:tr], -1.0, THRESHOLD, Op.mult, Op.add
                    )
                # ---- serial scan for this half: 2 DVE instrs per step ----
                for t in range(tl + 1, tr + 1):
                    nc.vector.scalar_tensor_tensor(
                        out=Bb[:, :, t],
                        in0=Bb[:, :, t - 1],
                        scalar=BETA,
                        in1=Sb[:, :, t - 1],
                        op0=Op.mult,
                        op1=Op.subtract,
                    )
                    nc.vector.tensor_tensor(
                        Sb[:, :, t], Bb[:, :, t], TH[:, :, t - 1], Op.is_gt
                    )
                # ---- epilogue for this half: mem = A + B, DMA out ----
                for o in range(OT):
                    nc.gpsimd.tensor_tensor(
                        M[:, o, tl:tr], A[:, o, tl:tr], Bb[:, o, tl + 1 : tr + 1], Op.add
                    )
                    nc.sync.dma_start(
                        spk_d[o * 128 : (o + 1) * 128, tl:tr], Sb[:, o, tl + 1 : tr + 1]
                    )
                    nc.sync.dma_start(
                        mem_d[o * 128 : (o + 1) * 128, tl:tr], M[:, o, tl:tr]
                    )
    nc.finalize()
    return nc


def _get_nc(mm_dtype_name: str):
    if mm_dtype_name not in _CACHE:
        _CACHE[mm_dtype_name] = _build_nc(mm_dtype_name)
    return _CACHE[mm_dtype_name]


def run(x, W, mm_dtype_name="split3", trace=False):
    import ml_dtypes

    from concourse.bass_utils import run_bass_kernel_spmd

    bf16 = ml_dtypes.bfloat16
    nc = _get_nc(mm_dtype_name)
    x = np.asarray(x, dtype=np.float32)
    W = np.asarray(W, dtype=np.float32)
    in_maps = []
    if mm_dtype_name in ("split3", "custom"):
        x_hi = x.astype(bf16)
        x_lo = (x - x_hi.astype(np.float32)).astype(bf16)
        xT = np.ascontiguousarray(
            np.stack([x_hi.T, x_lo.T], axis=0)
        )  # [2, N_IN, T] bf16
        W_hi = W.astype(bf16)
        W_lo = (W - W_hi.astype(np.float32)).astype(bf16)
        for c in range(N_CORES):
            sl = slice(c * O_SHARD, (c + 1) * O_SHARD)
            WTc = np.ascontiguousarray(np.stack([W_hi[sl].T, W_lo[sl].T], axis=0))
            in_maps.append({"xT": xT, "WT": WTc})
    else:
        xT = np.ascontiguousarray(x.T)  # [N_IN, T]
        for c in range(N_CORES):
            WTc = np.ascontiguousarray(W[c * O_SHARD : (c + 1) * O_SHARD, :].T)
            in_maps.append({"xT": xT, "WT": WTc})
    res = run_bass_kernel_spmd(nc, in_maps, core_ids=list(range(N_CORES)), trace=trace)
    spk = np.concatenate([r["spk"] for r in res.results], axis=0).T
    mem = np.concatenate([r["mem"] for r in res.results], axis=0).T
    return (
        np.ascontiguousarray(spk),
        np.ascontiguousarray(mem),
    ), res


def kernel(x, W):
    out, _ = run(x, W)
    return out



# revision 17
# speedup vs baseline: 1.4270x; 1.0781x over previous
"""Trainium2 Bass kernel: SNN Leaky-Integrate-and-Fire layer.

Computes, for x [T=1024, N_IN=4096] f32 and W [N_OUT=4096, N_IN=4096] f32:
    cur = x @ W.T                                   # [T, N_OUT]
    mem_t = 0.9*mem_{t-1} + cur_t - (mem_{t-1} > 1)  # scan over T
    spk_t = (mem_t > 1)
returning (spk_rec, mem_rec), both [T, N_OUT] f32.

Sharding: N_OUT split across 8 NeuronCores (512 neurons each). x is
replicated (each core reads the T-half slices it needs exactly once).

Device algorithm per core (neurons on partitions, time along free dim):
  cur[o, t] accumulated in PSUM via 256 matmuls (K=4096 in 32 tiles,
  O=512 in 4 tiles of 128 partitions, T=1024 in 2 half-banks of 512).
  The scan is decomposed as mem = A + B with
     A_t = 0.9*A_{t-1} + cur_t        (pure linear -> one HW
                                       tensor_tensor_scan per 512 steps)
     B_t = 0.9*B_{t-1} - s_{t-1}      (spike-coupled residual)
     s_t = (B_t > theta_t),  theta_t = 1 - A_t   (precomputed in bulk)
  so the serial part is only 2 small DVE instructions per timestep.
"""

import numpy as np

T = 1024
N_IN = 4096
N_OUT = 4096
N_CORES = 8
O_SHARD = N_OUT // N_CORES  # 512
KT = N_IN // 128  # 32 k-tiles
OT = O_SHARD // 128  # 4 o-tiles
BETA = 0.9
THRESHOLD = 1.0

_CACHE = {}

LIF_OP_NAME = "LIF_SCAN_ANT"


def _build_lif_uops():
    """DVE uop program for the LIF recurrence:

        m_i = beta*m_{i-1} + c_i - (m_{i-1} > 1)
        (beta = CONST_0/s0, init m = CONST_1/s1); out[i] = m_i, len = N.

    3 cycles/element, structured like the silicon-validated stock
    TENSOR_TENSOR_SCAN (seed -> bubbles -> consuming steady uop, state in
    a-flops read via NEXT_ALU_OUT_A one cycle after the write):

      E  (consume, cycle 3i):   blk0: w = SUB(c_i, blk1.a = r_{i-1})
                  (cycle 3i+1): blk1: m_i = ADD(w, blk2.a = u_i); out flop
                                blk2..7: BYPASS chain -> WR0_LO
      Bx (bubble, cycle 3i+2):  blk1: r_i = IS_GT(CURR_ALU_OUT = m_i, 1)
                                      -> out+a flop
                  (cycle 3i+3): blk2: u_{i+1} = MULT(CURR_ALU_OUT = m_i, beta)
                                      -> out+a flop
      By (bubble):              inert; gives blk2.a write one cycle to land
                                before E's blk1 read.

    Seed S0 (stock-seed clone) parks init m in blk1's out flop so the first
    Bx computes r/u from it. All cross-element state rides a-flops and
    CURR_ALU_OUT temporal reads; SRC_0 is only touched by the consuming uop."""
    from concourse.dve_uop import (
        ENABLE,
        AluInp,
        AluOp,
        InpSel,
        OutPath,
        OutSel,
        Trigger,
        UopConfig,
    )

    CNT, NONE, SRC_DONE = Trigger.COUNT, Trigger.NONE, Trigger.SRC_TENSOR_DONE

    # 0: S0 seed — blk1.out <- 0 (hard-wired ZERO; CONST_1 is not delivered
    # to the datapath on non-consuming uops on this firmware, so chunk
    # carry-in is handled by prepending m_prev as an input column instead)
    s0 = UopConfig()
    s0.enable_input(InpSel.CONST_1, 1)
    s0.datapath_config[0].enable_alu(AluOp.BYPASS, AluInp.PREV_DELAY_0)
    s0.datapath_config[1].pass_through_alu()
    s0.datapath_config[1].alu_out_a_enable = ENABLE
    # blk2's out flop seeds Bx's u = beta*m read (CURR_ALU_OUT at blk2)
    s0.datapath_config[2].pass_through_alu()
    s0.trigger = (CNT, NONE, NONE)
    s0.next_uop = (1, 0, 0)
    s0.repeat_count = 1

    # 1: Bx working bubble — r and u from the m in blk1's out flop
    bx = UopConfig()
    bx.enable_input(InpSel.ONE_F32, 2)  # -> delay_1 (threshold)
    bx.enable_input(InpSel.CONST_0, 3)  # -> delay_2 (beta)
    bx.datapath_config[0].pass_through_delay(1, 2)
    bx.datapath_config[1].enable_alu(
        AluOp.IS_GT, AluInp.CURR_ALU_OUT, AluInp.PREV_DELAY_1
    )
    bx.datapath_config[1].alu_out_a_enable = ENABLE
    bx.datapath_config[1].pass_through_delay(2)
    bx.datapath_config[2].enable_alu(
        AluOp.MULTIPLY, AluInp.CURR_ALU_OUT, AluInp.PREV_DELAY_2
    )
    bx.datapath_config[2].alu_out_a_enable = ENABLE
    bx.trigger = (CNT, NONE, NONE)
    bx.next_uop = (2, 0, 0)
    bx.repeat_count = 1

    # 2: By inert bubble
    by = UopConfig()
    by.trigger = (CNT, NONE, NONE)
    by.next_uop = (3, 0, 0)
    by.repeat_count = 1

    # 3: E steady — consumes c, emits m
    e = UopConfig()
    e.enable_input(InpSel.SRC_0, 0)
    e.require_inp0 = ENABLE
    e.datapath_config[0].enable_alu(
        AluOp.SUBTRACT, AluInp.PREV_ALU_OUT, AluInp.NEXT_ALU_OUT_A
    )
    e.datapath_config[1].enable_alu(
        AluOp.ADD, AluInp.PREV_ALU_OUT, AluInp.NEXT_ALU_OUT_A
    )
    for b in range(2, 8):
        e.datapath_config[b].pass_through_alu()
    e.enable_output(OutSel.ALU_OUT, OutPath.WR0_LO)
    e.trigger = (SRC_DONE, CNT, NONE)
    e.next_uop = (0, 1, 0)
    e.repeat_count = 1

    uops = [s0, bx, by, e]
    for u in uops:
        u.validate("v3")
    return uops


def _register_lif_op():
    import numpy as np_

    import concourse.dve_ops as dve_ops
    from concourse.dve_ops import DveOp
    from concourse.dve_spec import Spec, Src0
    from concourse.dve_uop import DveOpSpec

    if LIF_OP_NAME in dve_ops._SUB_OPCODE_FOR_NAME:
        return _CACHE["lif_op"]

    def _ref(in0, in1, c0, c1, c2):
        out = np_.empty((in0.shape[0], in0.shape[1]), np_.float32)
        m = np_.full((in0.shape[0],), c1, np_.float32)
        for t in range(in0.shape[1]):
            r = (m > 1.0).astype(np_.float32)
            m = np_.float32(c0) * m + (in0[:, t] - r)
            out[:, t] = m
        return out

    class _RawDveOp(DveOp):
        def compile(self, ver):
            assert ver == "v3", "LIF_SCAN_ANT is v3/TRN2-only"
            return DveOpSpec(
                name=self.name,
                opcode=dve_ops.get_dve_sub_opcode(self.name),
                uops=_build_lif_uops(),
                rd1_en=False,
            )

    op = _RawDveOp(
        name=LIF_OP_NAME,
        spec=Spec(body=Src0, reference=_ref),
        subdim=False,
        uops_sha={},
    )
    dve_ops.OPS.append(op)
    dve_ops._SUB_OPCODE_FOR_NAME[op.name] = (
        dve_ops._CUSTOM_DVE_ROW_BASE + len(dve_ops.OPS) - 1
    )
    dve_ops.CUSTOM_DVE_SPECS[op.name] = op.spec
    _CACHE["lif_op"] = op
    return op


def _build_nc(mm_dtype_name: str):
    import concourse.bacc as bacc
    import concourse.mybir as mybir
    from concourse.tile import TileContext

    F32 = mybir.dt.float32
    custom = mm_dtype_name in ("custom", "cf32r")
    split3 = mm_dtype_name in ("split3", "custom")
    MMDT = (
        mybir.dt.bfloat16
        if split3
        else mybir.dt.float32r
        if mm_dtype_name == "cf32r"
        else getattr(mybir.dt, mm_dtype_name)
    )
    Op = mybir.AluOpType
    lif_op = _register_lif_op() if custom else None

    nc = bacc.Bacc(target_bir_lowering=False)
    # split3: xT/WT carry [2, ...] leading dim = (hi, lo) bf16 parts.
    xshape = [2, N_IN, T] if split3 else [N_IN, T]
    wshape = [2, N_IN, O_SHARD] if split3 else [N_IN, O_SHARD]
    xT_d = nc.dram_tensor("xT", xshape, MMDT, kind="ExternalInput")
    WT_d = nc.dram_tensor("WT", wshape, MMDT, kind="ExternalInput")
    spk_d = nc.dram_tensor("spk", [O_SHARD, T], F32, kind="ExternalOutput")
    mem_d = nc.dram_tensor("mem", [O_SHARD, T], F32, kind="ExternalOutput")

    with TileContext(nc) as tc:
        with (
            tc.tile_pool(name="sb", bufs=1) as sb,
            tc.tile_pool(name="xs", bufs=8) as xs,
            tc.tile_pool(name="psp", bufs=1, space="PSUM") as psp,
        ):
            # All weights resident: [128, 2|1, KT, O_SHARD]; k-tile k holds
            # WT rows k*128..k*128+127 (i.e. W.T), so wt[:, h, k, o*128:...]
            # is directly the matmul stationary operand [K=128, M=128].
            # W streams on the Activation HWDGE ring so x DMAs (SP ring)
            # are not queued behind it.
            NH = 2 if split3 else 1
            if split3:
                wt_view = WT_d.rearrange("h (k p) o -> p h k o", p=128)
            else:
                wt_view = WT_d.rearrange("(k p) o -> p () k o", p=128)
            w_eng = nc.scalar if custom else nc.sync
            # one tile per k-tile DMA: dependency granularity matches the
            # transfer, so matmul k waits only on its own 256KB (1st at ~1us)
            wg = 1 if custom else 4
            wchunks = [
                sb.tile([128, NH, wg, O_SHARD], MMDT, name=f"wt{kc}")
                for kc in range(0, KT, wg)
            ]
            for h in range(NH):
                for ci, kc in enumerate(range(0, KT, wg)):
                    w_eng.dma_start(
                        wchunks[ci][:, h, :, :], wt_view[:, h, kc : kc + wg, :]
                    )

            def wt_sl(h, k, o):
                return wchunks[k // wg][:, h, k % wg, o * 128 : (o + 1) * 128]

            if custom:
                # one PSUM bank per (o, th) chunk so th1 matmuls never wait
                # on th0 scans through a shared-tile dependency
                psc = [
                    [
                        psp.tile([128, 512], F32, name=f"ps{o}_{th}", tag=f"ps{o}_{th}")
                        for th in range(2)
                    ]
                    for o in range(OT)
                ]
                ps = None
            else:
                ps = [
                    psp.tile([128, T], F32, name=f"ps{o}", tag=f"ps{o}")
                    for o in range(OT)
                ]

            if custom:
                M = sb.tile([128, OT, T], F32, name="M")
                Sp = sb.tile([128, OT, T], F32, name="Sp")
            else:
                A = sb.tile([128, OT, T], F32, name="A")  # linear-part scan
                TH = sb.tile([128, OT, T], F32, name="TH")  # theta = 1 - A
                M = sb.tile([128, OT, T], F32, name="M")  # mem = A + B
                Bb = sb.tile([128, OT, T + 1], F32, name="Bb")  # residual state
                Sb = sb.tile([128, OT, T + 1], F32, name="Sb")  # spikes (0/1)
                beta_t = sb.tile([128, 512], F32, name="beta_t")

                nc.vector.memset(beta_t, BETA)
                nc.vector.memset(Bb[:, :, 0], 0.0)
                nc.vector.memset(Sb[:, :, 0], 0.0)

            for th in range(2):
                tl, tr = th * 512, (th + 1) * 512
                # ---- matmul: accumulate cur[:, tl:tr] over all K ----
                for k in range(KT):
                    if split3:
                        xh = xs.tile([128, 2, 512], MMDT, name="xh")
                        nc.sync.dma_start(
                            xh,
                            xT_d[:, k * 128 : (k + 1) * 128, tl:tr].rearrange(
                                "h p t -> p h t"
                            ),
                        )
                        terms = [(0, 0), (1, 0), (0, 1)]  # (h_w, h_x)
                    else:
                        xh = xs.tile([128, 1, 512], MMDT, name="xh")
                        nc.sync.dma_start(
                            xh[:, 0, :], xT_d[k * 128 : (k + 1) * 128, tl:tr]
                        )
                        terms = [(0, 0)]
                    for o in range(OT):
                        tgt = psc[o][th] if custom else ps[o][:, tl:tr]
                        for ti, (hw, hx) in enumerate(terms):
                            nc.tensor.matmul(
                                tgt,
                                lhsT=wt_sl(hw, k, o),
                                rhs=xh[:, hx, :],
                                start=(k == 0 and ti == 0),
                                stop=(k == KT - 1 and ti == len(terms) - 1),
                            )
                if custom:
                    for o in range(OT):
                        init = 0.0 if th == 0 else M[:, o, tl - 1 : tl]
                        nc.vector._custom_dve(
                            lif_op,
                            out=M[:, o, tl:tr],
                            in0=psc[o][th],
                            s0=BETA,
                            s1=init,
                        )
                        nc.vector.tensor_scalar(
                            Sp[:, o, tl:tr],
                            M[:, o, tl:tr],
                            1.0,
                            None,
                            Op.is_gt,
                        )
                        nc.sync.dma_start(
                            spk_d[o * 128 : (o + 1) * 128, tl:tr], Sp[:, o, tl:tr]
                        )
                        nc.scalar.dma_start(
                            mem_d[o * 128 : (o + 1) * 128, tl:tr],
                            M[:, o, tl:tr],
                        )
                    continue
                # ---- bulk prep for this half: A scan + theta ----
                for o in range(OT):
                    init = 0.0 if th == 0 else A[:, o, tl - 1 : tl]
                    nc.vector.tensor_tensor_scan(
                        out=A[:, o, tl:tr],
                        data0=beta_t,
                        data1=ps[o][:, tl:tr],
                        initial=init,
                        op0=Op.mult,
                        op1=Op.add,
                    )
                    nc.gpsimd.tensor_scalar(
                        TH[:, o, tl:tr], A[:, o, tl:tr], -1.0, THRESHOLD, Op.mult, Op.add
                    )
                # ---- serial scan for this half: 2 DVE instrs per step ----
                for t in range(tl + 1, tr + 1):
                    nc.vector.scalar_tensor_tensor(
                        out=Bb[:, :, t],
                        in0=Bb[:, :, t - 1],
                        scalar=BETA,
                        in1=Sb[:, :, t - 1],
                        op0=Op.mult,
                        op1=Op.subtract,
                    )
                    nc.vector.tensor_tensor(
                        Sb[:, :, t], Bb[:, :, t], TH[:, :, t - 1], Op.is_gt
                    )
                # ---- epilogue for this half: mem = A + B, DMA out ----
                for o in range(OT):
                    nc.gpsimd.tensor_tensor(
                        M[:, o, tl:tr], A[:, o, tl:tr], Bb[:, o, tl + 1 : tr + 1], Op.add
                    )
                    nc.sync.dma_start(
                        spk_d[o * 128 : (o + 1) * 128, tl:tr], Sb[:, o, tl + 1 : tr + 1]
                    )
                    nc.sync.dma_start(
                        mem_d[o * 128 : (o + 1) * 128, tl:tr], M[:, o, tl:tr]
                    )
    nc.finalize()
    return nc


def _get_nc(mm_dtype_name: str):
    if mm_dtype_name not in _CACHE:
        _CACHE[mm_dtype_name] = _build_nc(mm_dtype_name)
    return _CACHE[mm_dtype_name]


def run(x, W, mm_dtype_name="split3", trace=False):
    import ml_dtypes

    from concourse.bass_utils import run_bass_kernel_spmd

    bf16 = ml_dtypes.bfloat16
    nc = _get_nc(mm_dtype_name)
    x = np.asarray(x, dtype=np.float32)
    W = np.asarray(W, dtype=np.float32)
    in_maps = []
    if mm_dtype_name in ("split3", "custom"):
        x_hi = x.astype(bf16)
        x_lo = (x - x_hi.astype(np.float32)).astype(bf16)
        xT = np.ascontiguousarray(
            np.stack([x_hi.T, x_lo.T], axis=0)
        )  # [2, N_IN, T] bf16
        W_hi = W.astype(bf16)
        W_lo = (W - W_hi.astype(np.float32)).astype(bf16)
        for c in range(N_CORES):
            sl = slice(c * O_SHARD, (c + 1) * O_SHARD)
            WTc = np.ascontiguousarray(np.stack([W_hi[sl].T, W_lo[sl].T], axis=0))
            in_maps.append({"xT": xT, "WT": WTc})
    else:
        xT = np.ascontiguousarray(x.T)  # [N_IN, T]
        for c in range(N_CORES):
            WTc = np.ascontiguousarray(W[c * O_SHARD : (c + 1) * O_SHARD, :].T)
            in_maps.append({"xT": xT, "WT": WTc})
    res = run_bass_kernel_spmd(nc, in_maps, core_ids=list(range(N_CORES)), trace=trace)
    spk = np.concatenate([r["spk"] for r in res.results], axis=0).T
    mem = np.concatenate([r["mem"] for r in res.results], axis=0).T
    return (
        np.ascontiguousarray(spk),
        np.ascontiguousarray(mem),
    ), res


def kernel(x, W):
    out, _ = run(x, W)
    return out



# revision 20
# speedup vs baseline: 1.4518x; 1.0174x over previous
"""Trainium2 Bass kernel: SNN Leaky-Integrate-and-Fire layer.

Computes, for x [T=1024, N_IN=4096] f32 and W [N_OUT=4096, N_IN=4096] f32:
    cur = x @ W.T                                   # [T, N_OUT]
    mem_t = 0.9*mem_{t-1} + cur_t - (mem_{t-1} > 1)  # scan over T
    spk_t = (mem_t > 1)
returning (spk_rec, mem_rec), both [T, N_OUT] f32.

Sharding: N_OUT split across 8 NeuronCores (512 neurons each). x is
replicated (each core reads the T-half slices it needs exactly once).

Device algorithm per core (neurons on partitions, time along free dim):
  cur[o, t] accumulated in PSUM via 256 matmuls (K=4096 in 32 tiles,
  O=512 in 4 tiles of 128 partitions, T=1024 in 2 half-banks of 512).
  The scan is decomposed as mem = A + B with
     A_t = 0.9*A_{t-1} + cur_t        (pure linear -> one HW
                                       tensor_tensor_scan per 512 steps)
     B_t = 0.9*B_{t-1} - s_{t-1}      (spike-coupled residual)
     s_t = (B_t > theta_t),  theta_t = 1 - A_t   (precomputed in bulk)
  so the serial part is only 2 small DVE instructions per timestep.
"""

import numpy as np

T = 1024
N_IN = 4096
N_OUT = 4096
N_CORES = 8
O_SHARD = N_OUT // N_CORES  # 512
KT = N_IN // 128  # 32 k-tiles
OT = O_SHARD // 128  # 4 o-tiles
BETA = 0.9
THRESHOLD = 1.0

_CACHE = {}

LIF_OP_NAME = "LIF_SCAN_ANT"


def _build_lif_uops():
    """DVE uop program for the LIF recurrence:

        m_i = beta*m_{i-1} + c_i - (m_{i-1} > 1)
        (beta = CONST_0/s0, init m = CONST_1/s1); out[i] = m_i, len = N.

    3 cycles/element, structured like the silicon-validated stock
    TENSOR_TENSOR_SCAN (seed -> bubbles -> consuming steady uop, state in
    a-flops read via NEXT_ALU_OUT_A one cycle after the write):

      E  (consume, cycle 3i):   blk0: w = SUB(c_i, blk1.a = r_{i-1})
                  (cycle 3i+1): blk1: m_i = ADD(w, blk2.a = u_i); out flop
                                blk2..7: BYPASS chain -> WR0_LO
      Bx (bubble, cycle 3i+2):  blk1: r_i = IS_GT(CURR_ALU_OUT = m_i, 1)
                                      -> out+a flop
                  (cycle 3i+3): blk2: u_{i+1} = MULT(CURR_ALU_OUT = m_i, beta)
                                      -> out+a flop
      By (bubble):              inert; gives blk2.a write one cycle to land
                                before E's blk1 read.

    Seed S0 (stock-seed clone) parks init m in blk1's out flop so the first
    Bx computes r/u from it. All cross-element state rides a-flops and
    CURR_ALU_OUT temporal reads; SRC_0 is only touched by the consuming uop."""
    from concourse.dve_uop import (
        ENABLE,
        AluInp,
        AluOp,
        InpSel,
        OutPath,
        OutSel,
        Trigger,
        UopConfig,
    )

    CNT, NONE, SRC_DONE = Trigger.COUNT, Trigger.NONE, Trigger.SRC_TENSOR_DONE

    # 0: S0 seed — blk1.out <- 0 (hard-wired ZERO; CONST_1 is not delivered
    # to the datapath on non-consuming uops on this firmware, so chunk
    # carry-in is handled by prepending m_prev as an input column instead)
    s0 = UopConfig()
    s0.enable_input(InpSel.CONST_1, 1)
    s0.datapath_config[0].enable_alu(AluOp.BYPASS, AluInp.PREV_DELAY_0)
    s0.datapath_config[1].pass_through_alu()
    s0.datapath_config[1].alu_out_a_enable = ENABLE
    # blk2's out flop seeds Bx's u = beta*m read (CURR_ALU_OUT at blk2)
    s0.datapath_config[2].pass_through_alu()
    s0.trigger = (CNT, NONE, NONE)
    s0.next_uop = (1, 0, 0)
    s0.repeat_count = 1

    # 1: Bx working bubble — r and u from the m in blk1's out flop
    bx = UopConfig()
    bx.enable_input(InpSel.ONE_F32, 2)  # -> delay_1 (threshold)
    bx.enable_input(InpSel.CONST_0, 3)  # -> delay_2 (beta)
    bx.datapath_config[0].pass_through_delay(1, 2)
    bx.datapath_config[1].enable_alu(
        AluOp.IS_GT, AluInp.CURR_ALU_OUT, AluInp.PREV_DELAY_1
    )
    bx.datapath_config[1].alu_out_a_enable = ENABLE
    bx.datapath_config[1].pass_through_delay(2)
    bx.datapath_config[2].enable_alu(
        AluOp.MULTIPLY, AluInp.CURR_ALU_OUT, AluInp.PREV_DELAY_2
    )
    bx.datapath_config[2].alu_out_a_enable = ENABLE
    bx.trigger = (CNT, NONE, NONE)
    bx.next_uop = (2, 0, 0)
    bx.repeat_count = 1

    # 2: By inert bubble
    by = UopConfig()
    by.trigger = (CNT, NONE, NONE)
    by.next_uop = (3, 0, 0)
    by.repeat_count = 1

    # 3: E steady — consumes c, emits m
    e = UopConfig()
    e.enable_input(InpSel.SRC_0, 0)
    e.require_inp0 = ENABLE
    e.datapath_config[0].enable_alu(
        AluOp.SUBTRACT, AluInp.PREV_ALU_OUT, AluInp.NEXT_ALU_OUT_A
    )
    e.datapath_config[1].enable_alu(
        AluOp.ADD, AluInp.PREV_ALU_OUT, AluInp.NEXT_ALU_OUT_A
    )
    for b in range(2, 8):
        e.datapath_config[b].pass_through_alu()
    e.enable_output(OutSel.ALU_OUT, OutPath.WR0_LO)
    e.trigger = (SRC_DONE, CNT, NONE)
    e.next_uop = (0, 1, 0)
    e.repeat_count = 1

    uops = [s0, bx, by, e]
    for u in uops:
        u.validate("v3")
    return uops


def _register_lif_op():
    import numpy as np_

    import concourse.dve_ops as dve_ops
    from concourse.dve_ops import DveOp
    from concourse.dve_spec import Spec, Src0
    from concourse.dve_uop import DveOpSpec

    if LIF_OP_NAME in dve_ops._SUB_OPCODE_FOR_NAME:
        return _CACHE["lif_op"]

    def _ref(in0, in1, c0, c1, c2):
        out = np_.empty((in0.shape[0], in0.shape[1]), np_.float32)
        m = np_.full((in0.shape[0],), c1, np_.float32)
        for t in range(in0.shape[1]):
            r = (m > 1.0).astype(np_.float32)
            m = np_.float32(c0) * m + (in0[:, t] - r)
            out[:, t] = m
        return out

    class _RawDveOp(DveOp):
        def compile(self, ver):
            assert ver == "v3", "LIF_SCAN_ANT is v3/TRN2-only"
            return DveOpSpec(
                name=self.name,
                opcode=dve_ops.get_dve_sub_opcode(self.name),
                uops=_build_lif_uops(),
                rd1_en=False,
            )

    op = _RawDveOp(
        name=LIF_OP_NAME,
        spec=Spec(body=Src0, reference=_ref),
        subdim=False,
        uops_sha={},
    )
    dve_ops.OPS.append(op)
    dve_ops._SUB_OPCODE_FOR_NAME[op.name] = (
        dve_ops._CUSTOM_DVE_ROW_BASE + len(dve_ops.OPS) - 1
    )
    dve_ops.CUSTOM_DVE_SPECS[op.name] = op.spec
    _CACHE["lif_op"] = op
    return op


def _build_nc(mm_dtype_name: str):
    import concourse.bacc as bacc
    import concourse.mybir as mybir
    from concourse.tile import TileContext

    F32 = mybir.dt.float32
    custom = mm_dtype_name in ("custom", "cf32r")
    split3 = mm_dtype_name in ("split3", "custom")
    MMDT = (
        mybir.dt.bfloat16
        if split3
        else mybir.dt.float32r
        if mm_dtype_name == "cf32r"
        else getattr(mybir.dt, mm_dtype_name)
    )
    Op = mybir.AluOpType
    lif_op = _register_lif_op() if custom else None

    nc = bacc.Bacc(target_bir_lowering=False)
    # split3: xT/WT carry [2, ...] leading dim = (hi, lo) bf16 parts.
    xshape = [2, N_IN, T] if split3 else [N_IN, T]
    wshape = [2, N_IN, O_SHARD] if split3 else [N_IN, O_SHARD]
    xT_d = nc.dram_tensor("xT", xshape, MMDT, kind="ExternalInput")
    WT_d = nc.dram_tensor("WT", wshape, MMDT, kind="ExternalInput")
    spk_d = nc.dram_tensor("spk", [O_SHARD, T], F32, kind="ExternalOutput")
    mem_d = nc.dram_tensor("mem", [O_SHARD, T], F32, kind="ExternalOutput")

    with TileContext(nc) as tc:
        with (
            tc.tile_pool(name="sb", bufs=1) as sb,
            tc.tile_pool(name="xs", bufs=4) as xs,
            tc.tile_pool(name="psp", bufs=1, space="PSUM") as psp,
        ):
            # All weights resident: [128, 2|1, KT, O_SHARD]; k-tile k holds
            # WT rows k*128..k*128+127 (i.e. W.T), so wt[:, h, k, o*128:...]
            # is directly the matmul stationary operand [K=128, M=128].
            # W streams on the Activation HWDGE ring so x DMAs (SP ring)
            # are not queued behind it.
            NH = 2 if split3 else 1
            if split3:
                wt_view = WT_d.rearrange("h (k p) o -> p h k o", p=128)
            else:
                wt_view = WT_d.rearrange("(k p) o -> p () k o", p=128)
            w_eng = nc.scalar if custom else nc.sync
            # one tile per W DMA: dependency granularity matches the
            # transfer (each dma_start costs ~600ns of engine issue time,
            # so fewer+larger transfers win once the first chunk is cheap)
            wg = 2 if custom else 4
            wchunks = [
                sb.tile([128, NH, wg, O_SHARD], MMDT, name=f"wt{kc}")
                for kc in range(0, KT, wg)
            ]
            for h in range(NH):
                for ci, kc in enumerate(range(0, KT, wg)):
                    w_eng.dma_start(
                        wchunks[ci][:, h, :, :], wt_view[:, h, kc : kc + wg, :]
                    )

            def wt_sl(h, k, o):
                return wchunks[k // wg][:, h, k % wg, o * 128 : (o + 1) * 128]

            if custom:
                # one PSUM bank per (o, th) chunk so th1 matmuls never wait
                # on th0 scans through a shared-tile dependency
                psc = [
                    [
                        psp.tile([128, 512], F32, name=f"ps{o}_{th}", tag=f"ps{o}_{th}")
                        for th in range(2)
                    ]
                    for o in range(OT)
                ]
                ps = None
            else:
                ps = [
                    psp.tile([128, T], F32, name=f"ps{o}", tag=f"ps{o}")
                    for o in range(OT)
                ]

            if custom:
                M = sb.tile([128, OT, T], F32, name="M")
                Sp = sb.tile([128, OT, T], F32, name="Sp")
            else:
                A = sb.tile([128, OT, T], F32, name="A")  # linear-part scan
                TH = sb.tile([128, OT, T], F32, name="TH")  # theta = 1 - A
                M = sb.tile([128, OT, T], F32, name="M")  # mem = A + B
                Bb = sb.tile([128, OT, T + 1], F32, name="Bb")  # residual state
                Sb = sb.tile([128, OT, T + 1], F32, name="Sb")  # spikes (0/1)
                beta_t = sb.tile([128, 512], F32, name="beta_t")

                nc.vector.memset(beta_t, BETA)
                nc.vector.memset(Bb[:, :, 0], 0.0)
                nc.vector.memset(Sb[:, :, 0], 0.0)

            XG = 4 if custom else 1  # k-tiles per x DMA
            for th in range(2):
                tl, tr = th * 512, (th + 1) * 512
                # ---- matmul: accumulate cur[:, tl:tr] over all K ----
                for kg in range(0, KT, XG):
                    if split3:
                        xh = xs.tile([128, 2, XG, 512], MMDT, name="xh")
                        nc.sync.dma_start(
                            xh,
                            xT_d[
                                :, kg * 128 : (kg + XG) * 128, tl:tr
                            ].rearrange("h (g p) t -> p h g t", p=128),
                        )
                        terms = [(0, 0), (1, 0), (0, 1)]  # (h_w, h_x)
                    else:
                        xh = xs.tile([128, 1, XG, 512], MMDT, name="xh")
                        nc.sync.dma_start(
                            xh[:, 0, :, :],
                            xT_d[kg * 128 : (kg + XG) * 128, tl:tr].rearrange(
                                "(g p) t -> p g t", p=128
                            ),
                        )
                        terms = [(0, 0)]
                    for kk in range(XG):
                        k = kg + kk
                        for o in range(OT):
                            tgt = psc[o][th] if custom else ps[o][:, tl:tr]
                            for ti, (hw, hx) in enumerate(terms):
                                nc.tensor.matmul(
                                    tgt,
                                    lhsT=wt_sl(hw, k, o),
                                    rhs=xh[:, hx, kk, :],
                                    start=(k == 0 and ti == 0),
                                    stop=(k == KT - 1 and ti == len(terms) - 1),
                                )
                if custom:
                    for o in range(OT):
                        init = 0.0 if th == 0 else M[:, o, tl - 1 : tl]
                        nc.vector._custom_dve(
                            lif_op,
                            out=M[:, o, tl:tr],
                            in0=psc[o][th],
                            s0=BETA,
                            s1=init,
                        )
                        nc.vector.tensor_scalar(
                            Sp[:, o, tl:tr],
                            M[:, o, tl:tr],
                            1.0,
                            None,
                            Op.is_gt,
                        )
                        nc.sync.dma_start(
                            spk_d[o * 128 : (o + 1) * 128, tl:tr], Sp[:, o, tl:tr]
                        )
                        nc.scalar.dma_start(
                            mem_d[o * 128 : (o + 1) * 128, tl:tr],
                            M[:, o, tl:tr],
                        )
                    continue
                # ---- bulk prep for this half: A scan + theta ----
                for o in range(OT):
                    init = 0.0 if th == 0 else A[:, o, tl - 1 : tl]
                    nc.vector.tensor_tensor_scan(
                        out=A[:, o, tl:tr],
                        data0=beta_t,
                        data1=ps[o][:, tl:tr],
                        initial=init,
                        op0=Op.mult,
                        op1=Op.add,
                    )
                    nc.gpsimd.tensor_scalar(
                        TH[:, o, tl:tr], A[:, o, tl:tr], -1.0, THRESHOLD, Op.mult, Op.add
                    )
                # ---- serial scan for this half: 2 DVE instrs per step ----
                for t in range(tl + 1, tr + 1):
                    nc.vector.scalar_tensor_tensor(
                        out=Bb[:, :, t],
                        in0=Bb[:, :, t - 1],
                        scalar=BETA,
                        in1=Sb[:, :, t - 1],
                        op0=Op.mult,
                        op1=Op.subtract,
                    )
                    nc.vector.tensor_tensor(
                        Sb[:, :, t], Bb[:, :, t], TH[:, :, t - 1], Op.is_gt
                    )
                # ---- epilogue for this half: mem = A + B, DMA out ----
                for o in range(OT):
                    nc.gpsimd.tensor_tensor(
                        M[:, o, tl:tr], A[:, o, tl:tr], Bb[:, o, tl + 1 : tr + 1], Op.add
                    )
                    nc.sync.dma_start(
                        spk_d[o * 128 : (o + 1) * 128, tl:tr], Sb[:, o, tl + 1 : tr + 1]
                    )
                    nc.sync.dma_start(
                        mem_d[o * 128 : (o + 1) * 128, tl:tr], M[:, o, tl:tr]
                    )
    nc.finalize()
    return nc


def _get_nc(mm_dtype_name: str):
    if mm_dtype_name not in _CACHE:
        _CACHE[mm_dtype_name] = _build_nc(mm_dtype_name)
    return _CACHE[mm_dtype_name]


def run(x, W, mm_dtype_name="split3", trace=False):
    import ml_dtypes

    from concourse.bass_utils import run_bass_kernel_spmd

    bf16 = ml_dtypes.bfloat16
    nc = _get_nc(mm_dtype_name)
    x = np.asarray(x, dtype=np.float32)
    W = np.asarray(W, dtype=np.float32)
    in_maps = []
    if mm_dtype_name in ("split3", "custom"):
        x_hi = x.astype(bf16)
        x_lo = (x - x_hi.astype(np.float32)).astype(bf16)
        xT = np.ascontiguousarray(
            np.stack([x_hi.T, x_lo.T], axis=0)
        )  # [2, N_IN, T] bf16
        W_hi = W.astype(bf16)
        W_lo = (W - W_hi.astype(np.float32)).astype(bf16)
        for c in range(N_CORES):
            sl = slice(c * O_SHARD, (c + 1) * O_SHARD)
            WTc = np.ascontiguousarray(np.stack([W_hi[sl].T, W_lo[sl].T], axis=0))
            in_maps.append({"xT": xT, "WT": WTc})
    else:
        xT = np.ascontiguousarray(x.T)  # [N_IN, T]
        for c in range(N_CORES):
            WTc = np.ascontiguousarray(W[c * O_SHARD : (c + 1) * O_SHARD, :].T)
            in_maps.append({"xT": xT, "WT": WTc})
    res = run_bass_kernel_spmd(nc, in_maps, core_ids=list(range(N_CORES)), trace=trace)
    spk = np.concatenate([r["spk"] for r in res.results], axis=0).T
    mem = np.concatenate([r["mem"] for r in res.results], axis=0).T
    return (
        np.ascontiguousarray(spk),
        np.ascontiguousarray(mem),
    ), res


def kernel(x, W):
    out, _ = run(x, W)
    return out



# revision 24
# speedup vs baseline: 1.5472x; 1.0657x over previous
"""Trainium2 Bass kernel: SNN Leaky-Integrate-and-Fire layer.

Computes, for x [T=1024, N_IN=4096] f32 and W [N_OUT=4096, N_IN=4096] f32:
    cur = x @ W.T                                   # [T, N_OUT]
    mem_t = 0.9*mem_{t-1} + cur_t - (mem_{t-1} > 1)  # scan over T
    spk_t = (mem_t > 1)
returning (spk_rec, mem_rec), both [T, N_OUT] f32.

Sharding: N_OUT split across 8 NeuronCores (512 neurons each). x is
replicated (each core reads the T-half slices it needs exactly once).

Device algorithm per core (neurons on partitions, time along free dim):
  cur[o, t] accumulated in PSUM via 256 matmuls (K=4096 in 32 tiles,
  O=512 in 4 tiles of 128 partitions, T=1024 in 2 half-banks of 512).
  The scan is decomposed as mem = A + B with
     A_t = 0.9*A_{t-1} + cur_t        (pure linear -> one HW
                                       tensor_tensor_scan per 512 steps)
     B_t = 0.9*B_{t-1} - s_{t-1}      (spike-coupled residual)
     s_t = (B_t > theta_t),  theta_t = 1 - A_t   (precomputed in bulk)
  so the serial part is only 2 small DVE instructions per timestep.
"""

import numpy as np

T = 1024
N_IN = 4096
N_OUT = 4096
N_CORES = 8
O_SHARD = N_OUT // N_CORES  # 512
KT = N_IN // 128  # 32 k-tiles
OT = O_SHARD // 128  # 4 o-tiles
BETA = 0.9
THRESHOLD = 1.0

_CACHE = {}

LIF_OP_NAME = "LIF_SCAN_ANT"


def _build_lif_uops():
    """DVE uop program for the LIF recurrence:

        m_i = beta*m_{i-1} + c_i - (m_{i-1} > 1)
        (beta = CONST_0/s0, init m = CONST_1/s1); out[i] = m_i, len = N.

    3 cycles/element, structured like the silicon-validated stock
    TENSOR_TENSOR_SCAN (seed -> bubbles -> consuming steady uop, state in
    a-flops read via NEXT_ALU_OUT_A one cycle after the write):

      E  (consume, cycle 3i):   blk0: w = SUB(c_i, blk1.a = r_{i-1})
                  (cycle 3i+1): blk1: m_i = ADD(w, blk2.a = u_i); out flop
                                blk2..7: BYPASS chain -> WR0_LO
      Bx (bubble, cycle 3i+2):  blk1: r_i = IS_GT(CURR_ALU_OUT = m_i, 1)
                                      -> out+a flop
                  (cycle 3i+3): blk2: u_{i+1} = MULT(CURR_ALU_OUT = m_i, beta)
                                      -> out+a flop
      By (bubble):              inert; gives blk2.a write one cycle to land
                                before E's blk1 read.

    Seed S0 (stock-seed clone) parks init m in blk1's out flop so the first
    Bx computes r/u from it. All cross-element state rides a-flops and
    CURR_ALU_OUT temporal reads; SRC_0 is only touched by the consuming uop."""
    from concourse.dve_uop import (
        ENABLE,
        AluInp,
        AluOp,
        InpSel,
        OutPath,
        OutSel,
        Trigger,
        UopConfig,
    )

    CNT, NONE, SRC_DONE = Trigger.COUNT, Trigger.NONE, Trigger.SRC_TENSOR_DONE

    # 0: S0 seed — blk1.out <- 0 (hard-wired ZERO; CONST_1 is not delivered
    # to the datapath on non-consuming uops on this firmware, so chunk
    # carry-in is handled by prepending m_prev as an input column instead)
    s0 = UopConfig()
    s0.enable_input(InpSel.CONST_1, 1)
    s0.datapath_config[0].enable_alu(AluOp.BYPASS, AluInp.PREV_DELAY_0)
    s0.datapath_config[1].pass_through_alu()
    s0.datapath_config[1].alu_out_a_enable = ENABLE
    # blk2's out flop seeds Bx's u = beta*m read (CURR_ALU_OUT at blk2)
    s0.datapath_config[2].pass_through_alu()
    s0.trigger = (CNT, NONE, NONE)
    s0.next_uop = (1, 0, 0)
    s0.repeat_count = 1

    # 1: Bx working bubble — r and u from the m in blk1's out flop
    bx = UopConfig()
    bx.enable_input(InpSel.ONE_F32, 2)  # -> delay_1 (threshold)
    bx.enable_input(InpSel.CONST_0, 3)  # -> delay_2 (beta)
    bx.datapath_config[0].pass_through_delay(1, 2)
    bx.datapath_config[1].enable_alu(
        AluOp.IS_GT, AluInp.CURR_ALU_OUT, AluInp.PREV_DELAY_1
    )
    bx.datapath_config[1].alu_out_a_enable = ENABLE
    bx.datapath_config[1].pass_through_delay(2)
    bx.datapath_config[2].enable_alu(
        AluOp.MULTIPLY, AluInp.CURR_ALU_OUT, AluInp.PREV_DELAY_2
    )
    bx.datapath_config[2].alu_out_a_enable = ENABLE
    bx.trigger = (CNT, NONE, NONE)
    bx.next_uop = (2, 0, 0)
    bx.repeat_count = 1

    # 2: By inert bubble
    by = UopConfig()
    by.trigger = (CNT, NONE, NONE)
    by.next_uop = (3, 0, 0)
    by.repeat_count = 1

    # 3: E steady — consumes c, emits m
    e = UopConfig()
    e.enable_input(InpSel.SRC_0, 0)
    e.require_inp0 = ENABLE
    e.datapath_config[0].enable_alu(
        AluOp.SUBTRACT, AluInp.PREV_ALU_OUT, AluInp.NEXT_ALU_OUT_A
    )
    e.datapath_config[1].enable_alu(
        AluOp.ADD, AluInp.PREV_ALU_OUT, AluInp.NEXT_ALU_OUT_A
    )
    for b in range(2, 8):
        e.datapath_config[b].pass_through_alu()
    e.enable_output(OutSel.ALU_OUT, OutPath.WR0_LO)
    e.trigger = (SRC_DONE, CNT, NONE)
    e.next_uop = (0, 1, 0)
    e.repeat_count = 1

    uops = [s0, bx, by, e]
    for u in uops:
        u.validate("v3")
    return uops


def _register_lif_op():
    import numpy as np_

    import concourse.dve_ops as dve_ops
    from concourse.dve_ops import DveOp
    from concourse.dve_spec import Spec, Src0
    from concourse.dve_uop import DveOpSpec

    if LIF_OP_NAME in dve_ops._SUB_OPCODE_FOR_NAME:
        return _CACHE["lif_op"]

    def _ref(in0, in1, c0, c1, c2):
        out = np_.empty((in0.shape[0], in0.shape[1]), np_.float32)
        m = np_.full((in0.shape[0],), c1, np_.float32)
        for t in range(in0.shape[1]):
            r = (m > 1.0).astype(np_.float32)
            m = np_.float32(c0) * m + (in0[:, t] - r)
            out[:, t] = m
        return out

    class _RawDveOp(DveOp):
        def compile(self, ver):
            assert ver == "v3", "LIF_SCAN_ANT is v3/TRN2-only"
            return DveOpSpec(
                name=self.name,
                opcode=dve_ops.get_dve_sub_opcode(self.name),
                uops=_build_lif_uops(),
                rd1_en=False,
            )

    op = _RawDveOp(
        name=LIF_OP_NAME,
        spec=Spec(body=Src0, reference=_ref),
        subdim=False,
        uops_sha={},
    )
    dve_ops.OPS.append(op)
    dve_ops._SUB_OPCODE_FOR_NAME[op.name] = (
        dve_ops._CUSTOM_DVE_ROW_BASE + len(dve_ops.OPS) - 1
    )
    dve_ops.CUSTOM_DVE_SPECS[op.name] = op.spec
    _CACHE["lif_op"] = op
    return op


def _build_nc(mm_dtype_name: str):
    import concourse.bacc as bacc
    import concourse.mybir as mybir
    from concourse.tile import TileContext

    F32 = mybir.dt.float32
    custom = mm_dtype_name in ("custom", "cf32r")
    split3 = mm_dtype_name in ("split3", "custom")
    MMDT = (
        mybir.dt.bfloat16
        if split3
        else mybir.dt.float32r
        if mm_dtype_name == "cf32r"
        else getattr(mybir.dt, mm_dtype_name)
    )
    Op = mybir.AluOpType
    lif_op = _register_lif_op() if custom else None

    fastx = mm_dtype_name == "cf32r"
    nc = bacc.Bacc(target_bir_lowering=False)
    # split3: xT/WT carry [2, ...] leading dim = (hi, lo) bf16 parts.
    # fastx: partition-major layouts so every partition line is 8KB+
    # contiguous: xT [128, 2, KT, 512] (p, th, k, t'), WT [128, KT, 512].
    if fastx:
        xshape = [128, 2, KT, 512]
        wshape = [128, KT, O_SHARD]
    else:
        xshape = [2, N_IN, T] if split3 else [N_IN, T]
        wshape = [2, N_IN, O_SHARD] if split3 else [N_IN, O_SHARD]
    xT_d = nc.dram_tensor("xT", xshape, MMDT, kind="ExternalInput")
    WT_d = nc.dram_tensor("WT", wshape, MMDT, kind="ExternalInput")
    spk_d = nc.dram_tensor("spk", [O_SHARD, T], F32, kind="ExternalOutput")
    mem_d = nc.dram_tensor("mem", [O_SHARD, T], F32, kind="ExternalOutput")

    with TileContext(nc) as tc:
        with (
            tc.tile_pool(name="sb", bufs=1) as sb,
            tc.tile_pool(name="xs", bufs=4) as xs,
            tc.tile_pool(name="psp", bufs=1, space="PSUM") as psp,
        ):
            # All weights resident: [128, 2|1, KT, O_SHARD]; k-tile k holds
            # WT rows k*128..k*128+127 (i.e. W.T), so wt[:, h, k, o*128:...]
            # is directly the matmul stationary operand [K=128, M=128].
            # W streams on the Activation HWDGE ring so x DMAs (SP ring)
            # are not queued behind it.
            NH = 2 if split3 else 1
            if fastx:
                wt_view = WT_d.rearrange("p k o -> p () k o")
            elif split3:
                wt_view = WT_d.rearrange("h (k p) o -> p h k o", p=128)
            else:
                wt_view = WT_d.rearrange("(k p) o -> p () k o", p=128)
            w_eng = nc.scalar if custom else nc.sync
            # one tile per W DMA: dependency granularity matches the
            # transfer (each dma_start costs ~600ns of engine issue time,
            # so fewer+larger transfers win once the first chunk is cheap)
            wg = 4 if custom else 4
            wchunks = [
                sb.tile([128, NH, wg, O_SHARD], MMDT, name=f"wt{kc}")
                for kc in range(0, KT, wg)
            ]
            for h in range(NH):
                for ci, kc in enumerate(range(0, KT, wg)):
                    w_eng.dma_start(
                        wchunks[ci][:, h, :, :], wt_view[:, h, kc : kc + wg, :]
                    )

            def wt_sl(h, k, o):
                return wchunks[k // wg][:, h, k % wg, o * 128 : (o + 1) * 128]

            if custom:
                # one PSUM bank per (o, th) chunk so th1 matmuls never wait
                # on th0 scans through a shared-tile dependency
                psc = [
                    [
                        psp.tile([128, 512], F32, name=f"ps{o}_{th}", tag=f"ps{o}_{th}")
                        for th in range(2)
                    ]
                    for o in range(OT)
                ]
                ps = None
            else:
                ps = [
                    psp.tile([128, T], F32, name=f"ps{o}", tag=f"ps{o}")
                    for o in range(OT)
                ]

            if custom:
                M = sb.tile([128, OT, T], F32, name="M")
                Sp = sb.tile([128, OT, T], F32, name="Sp")
            else:
                A = sb.tile([128, OT, T], F32, name="A")  # linear-part scan
                TH = sb.tile([128, OT, T], F32, name="TH")  # theta = 1 - A
                M = sb.tile([128, OT, T], F32, name="M")  # mem = A + B
                Bb = sb.tile([128, OT, T + 1], F32, name="Bb")  # residual state
                Sb = sb.tile([128, OT, T + 1], F32, name="Sb")  # spikes (0/1)
                beta_t = sb.tile([128, 512], F32, name="beta_t")

                nc.vector.memset(beta_t, BETA)
                nc.vector.memset(Bb[:, :, 0], 0.0)
                nc.vector.memset(Sb[:, :, 0], 0.0)

            XG = 4 if custom else 1  # k-tiles per x DMA
            for th in range(2):
                tl, tr = th * 512, (th + 1) * 512
                # ---- matmul: accumulate cur[:, tl:tr] over all K ----
                # th0 x on the SP ring; th1 x on the Activation ring, which
                # is idle once W has streamed in
                x_eng = nc.scalar if (fastx and th == 1) else nc.sync
                for kg in range(0, KT, XG):
                    if fastx:
                        xh = xs.tile([128, 1, XG, 512], MMDT, name="xh")
                        x_eng.dma_start(
                            xh[:, 0, :, :], xT_d[:, th, kg : kg + XG, :]
                        )
                        terms = [(0, 0)]
                    elif split3:
                        xh = xs.tile([128, 2, XG, 512], MMDT, name="xh")
                        nc.sync.dma_start(
                            xh,
                            xT_d[
                                :, kg * 128 : (kg + XG) * 128, tl:tr
                            ].rearrange("h (g p) t -> p h g t", p=128),
                        )
                        terms = [(0, 0), (1, 0), (0, 1)]  # (h_w, h_x)
                    else:
                        xh = xs.tile([128, 1, XG, 512], MMDT, name="xh")
                        nc.sync.dma_start(
                            xh[:, 0, :, :],
                            xT_d[kg * 128 : (kg + XG) * 128, tl:tr].rearrange(
                                "(g p) t -> p g t", p=128
                            ),
                        )
                        terms = [(0, 0)]
                    for kk in range(XG):
                        k = kg + kk
                        for o in range(OT):
                            tgt = psc[o][th] if custom else ps[o][:, tl:tr]
                            for ti, (hw, hx) in enumerate(terms):
                                nc.tensor.matmul(
                                    tgt,
                                    lhsT=wt_sl(hw, k, o),
                                    rhs=xh[:, hx, kk, :],
                                    start=(k == 0 and ti == 0),
                                    stop=(k == KT - 1 and ti == len(terms) - 1),
                                )
                if custom:
                    for o in range(OT):
                        init = 0.0 if th == 0 else M[:, o, tl - 1 : tl]
                        nc.vector._custom_dve(
                            lif_op,
                            out=M[:, o, tl:tr],
                            in0=psc[o][th],
                            s0=BETA,
                            s1=init,
                        )
                        nc.vector.tensor_scalar(
                            Sp[:, o, tl:tr],
                            M[:, o, tl:tr],
                            1.0,
                            None,
                            Op.is_gt,
                        )
                        nc.sync.dma_start(
                            spk_d[o * 128 : (o + 1) * 128, tl:tr], Sp[:, o, tl:tr]
                        )
                        nc.scalar.dma_start(
                            mem_d[o * 128 : (o + 1) * 128, tl:tr],
                            M[:, o, tl:tr],
                        )
                    continue
                # ---- bulk prep for this half: A scan + theta ----
                for o in range(OT):
                    init = 0.0 if th == 0 else A[:, o, tl - 1 : tl]
                    nc.vector.tensor_tensor_scan(
                        out=A[:, o, tl:tr],
                        data0=beta_t,
                        data1=ps[o][:, tl:tr],
                        initial=init,
                        op0=Op.mult,
                        op1=Op.add,
                    )
                    nc.gpsimd.tensor_scalar(
                        TH[:, o, tl:tr], A[:, o, tl:tr], -1.0, THRESHOLD, Op.mult, Op.add
                    )
                # ---- serial scan for this half: 2 DVE instrs per step ----
                for t in range(tl + 1, tr + 1):
                    nc.vector.scalar_tensor_tensor(
                        out=Bb[:, :, t],
                        in0=Bb[:, :, t - 1],
                        scalar=BETA,
                        in1=Sb[:, :, t - 1],
                        op0=Op.mult,
                        op1=Op.subtract,
                    )
                    nc.vector.tensor_tensor(
                        Sb[:, :, t], Bb[:, :, t], TH[:, :, t - 1], Op.is_gt
                    )
                # ---- epilogue for this half: mem = A + B, DMA out ----
                for o in range(OT):
                    nc.gpsimd.tensor_tensor(
                        M[:, o, tl:tr], A[:, o, tl:tr], Bb[:, o, tl + 1 : tr + 1], Op.add
                    )
                    nc.sync.dma_start(
                        spk_d[o * 128 : (o + 1) * 128, tl:tr], Sb[:, o, tl + 1 : tr + 1]
                    )
                    nc.sync.dma_start(
                        mem_d[o * 128 : (o + 1) * 128, tl:tr], M[:, o, tl:tr]
                    )
    nc.finalize()
    return nc


def _get_nc(mm_dtype_name: str):
    if mm_dtype_name not in _CACHE:
        _CACHE[mm_dtype_name] = _build_nc(mm_dtype_name)
    return _CACHE[mm_dtype_name]


def run(x, W, mm_dtype_name="split3", trace=False):
    import ml_dtypes

    from concourse.bass_utils import run_bass_kernel_spmd

    bf16 = ml_dtypes.bfloat16
    nc = _get_nc(mm_dtype_name)
    x = np.asarray(x, dtype=np.float32)
    W = np.asarray(W, dtype=np.float32)
    in_maps = []
    if mm_dtype_name in ("split3", "custom"):
        x_hi = x.astype(bf16)
        x_lo = (x - x_hi.astype(np.float32)).astype(bf16)
        xT = np.ascontiguousarray(
            np.stack([x_hi.T, x_lo.T], axis=0)
        )  # [2, N_IN, T] bf16
        W_hi = W.astype(bf16)
        W_lo = (W - W_hi.astype(np.float32)).astype(bf16)
        for c in range(N_CORES):
            sl = slice(c * O_SHARD, (c + 1) * O_SHARD)
            WTc = np.ascontiguousarray(np.stack([W_hi[sl].T, W_lo[sl].T], axis=0))
            in_maps.append({"xT": xT, "WT": WTc})
    elif mm_dtype_name == "cf32r":
        # x3[p, th, k, t'] = x[th*512+t', k*128+p]; per-partition lines are
        # 8KB-contiguous for grouped k-tile DMA reads
        x3 = np.ascontiguousarray(
            x.reshape(2, 512, KT, 128).transpose(3, 0, 2, 1)
        )  # [128, 2, KT, 512]
        for c in range(N_CORES):
            WTc = W[c * O_SHARD : (c + 1) * O_SHARD, :].T  # [N_IN, 512]
            W3c = np.ascontiguousarray(
                WTc.reshape(KT, 128, O_SHARD).transpose(1, 0, 2)
            )  # [128, KT, 512]
            in_maps.append({"xT": x3, "WT": W3c})
    else:
        xT = np.ascontiguousarray(x.T)  # [N_IN, T]
        for c in range(N_CORES):
            WTc = np.ascontiguousarray(W[c * O_SHARD : (c + 1) * O_SHARD, :].T)
            in_maps.append({"xT": xT, "WT": WTc})
    res = run_bass_kernel_spmd(nc, in_maps, core_ids=list(range(N_CORES)), trace=trace)
    spk = np.concatenate([r["spk"] for r in res.results], axis=0).T
    mem = np.concatenate([r["mem"] for r in res.results], axis=0).T
    return (
        np.ascontiguousarray(spk),
        np.ascontiguousarray(mem),
    ), res


def kernel(x, W):
    out, _ = run(x, W)
    return out

